# revision 15
# baseline (speedup 1.0000x reference)
"""AttentivePooling Trainium2 kernel.

Reference semantics (h_all: [T, B, D] f32, xin unused):
    h_last = h_all[-1]                       # [B, D]
    a[b, t] = <h_all[t, b, :], h_last[b, :]> / sqrt(D)
    r = relu(a)
    w = r / (sum_t r + 1e-9)
    out[b, d] = sum_t w[b, t] * h_all[t, b, d]

Strategy: data-parallel over B across 8 cores (8 batches/core, no
collectives). Per batch on-device:
  - one 4MB DMA loads h_b as 16 SBUF chunks [128(t), 512(d)] (t = c*128+p)
  - PE broadcasts h_last across 128 partitions (ones[1,128].T @ hl[1,512])
  - DVE tensor_tensor_reduce computes per-t dot products (scores) fused
    with the 1/sqrt(D) scale
  - ACT relu with accum_out produces weights + their per-partition sums
  - PE accumulates sum_t w_t * h_t into PSUM [1, 512] (w as stationary)
  - PE reduces the weight-sum across partitions via ones column
  - DVE computes 1/(Z + 1e-9) and scales the pooled vector
"""

import numpy as np
from contextlib import ExitStack

import concourse.bass as bass
import concourse.tile as tile
from concourse import bacc, mybir
from concourse.bass_utils import run_bass_kernel_spmd

T, B, D = 2048, 64, 512
NCORES = 8
BPC = B // NCORES  # batches per core
P = 128
TC = T // P  # 16 T-chunks per batch
SCALE = float(1.0 / np.sqrt(np.float32(D)))

_nc_cache = None


def _build():
    global _nc_cache
    if _nc_cache is not None:
        return _nc_cache
    nc = bacc.Bacc("TRN2", debug=False, target_bir_lowering=False, num_devices=NCORES)
    h = nc.dram_tensor("h", [T, BPC, D], mybir.dt.float32, kind="ExternalInput")
    out = nc.dram_tensor("out", [BPC, D], mybir.dt.float32, kind="ExternalOutput")
    h_ap = h.ap()
    out_ap = out.ap()
    f32 = mybir.dt.float32

    with tile.TileContext(nc) as tc:
        with ExitStack() as ctx:
            hpool = ctx.enter_context(tc.tile_pool(name="h", bufs=3))
            hlbp = ctx.enter_context(tc.tile_pool(name="hlb", bufs=2))
            tmpp = ctx.enter_context(tc.tile_pool(name="tmp", bufs=3))
            smallp = ctx.enter_context(tc.tile_pool(name="small", bufs=2))
            constp = ctx.enter_context(tc.tile_pool(name="const", bufs=1))
            psbcp = ctx.enter_context(tc.tile_pool(name="psb", bufs=2, space="PSUM"))
            psonep = ctx.enter_context(tc.tile_pool(name="psone", bufs=1, space="PSUM"))
            psoutp = ctx.enter_context(tc.tile_pool(name="pso", bufs=2, space="PSUM"))
            pszp = ctx.enter_context(tc.tile_pool(name="psz", bufs=2, space="PSUM"))

            ones_row = constp.tile([1, P], f32)
            nc.vector.memset(ones_row[:], 1.0)
            eps_tile = constp.tile([1, 1], f32)
            nc.vector.memset(eps_tile[:], 1e-9)
            # ones_col generated on PE so later matmuls never need a
            # DVE-sem wait (matmul instructions fit only one sync wait)
            ps_ones = psonep.tile([P, 1], f32)
            mm_ones = nc.tensor.matmul(
                ps_ones[:], ones_row[:, :], ones_row[0:1, 0:1], start=True, stop=True
            )
            ones_col = constp.tile([P, 1], f32)
            nc.scalar.copy(ones_col[:], ps_ones[:])
            # all h_last rows for this core's batches, on partition 0
            hl_flat = constp.tile([1, BPC * D], f32)
            nc.sync.dma_start(
                hl_flat[:], h_ap[T - 1 : T, :, :].rearrange("t b d -> t (b d)")
            )

            prev_pe = mm_ones
            prev_relu = None
            prev_zeps = None
            prev_res = None
            for b in range(BPC):
                h_sb = hpool.tile([P, TC, D], f32, tag="hsb")
                # SWDGE: a single HWDGE dma_start with 2048 descriptors
                # wedges the exec unit; gpsimd handles it fine.
                nc.gpsimd.dma_start(
                    h_sb[:], h_ap[:, b, :].rearrange("(c p) d -> p c d", p=P)
                )

                # broadcast h_last[b] to all 128 partitions
                psb = psbcp.tile([P, D], f32, tag="psb")
                mm_b = nc.tensor.matmul(
                    psb[:],
                    ones_row[:, :],
                    hl_flat[0:1, b * D : (b + 1) * D],
                    start=True,
                    stop=True,
                )
                bass._add_dep_helper(
                    mm_b.ins, prev_pe.ins, sync=False, reason="pe order"
                )
                hlb = hlbp.tile([P, D], f32, tag="hlb")
                nc.scalar.copy(hlb[:], psb[:])

                # scores: scr[p, c] = sum_d h[t, d] * hl[d] * SCALE.
                # DVE does the elementwise product; ACT does the free-dim
                # reduction via activation accum_out (the fused DVE
                # tensor_tensor_reduce crashes the exec unit on this HW).
                scr = smallp.tile([P, TC], f32, tag="scr")
                for c in range(TC):
                    tmp = tmpp.tile([P, D], f32, tag="tmp")
                    nc.vector.tensor_tensor(
                        tmp[:], h_sb[:, c, :], hlb[:], mybir.AluOpType.mult
                    )
                    nc.scalar.activation(
                        tmp[:],
                        tmp[:],
                        mybir.ActivationFunctionType.Copy,
                        scale=SCALE,
                        accum_out=scr[:, c : c + 1],
                    )

                # relu + per-partition sum of relu'd scores.  Force ACT
                # program order: prior batch's PSUM-release readers come
                # before this relu, so matmul waits on the ACT sem merge
                # into a single wait at relu's tick.
                w = smallp.tile([P, TC], f32, tag="w")
                zcol = smallp.tile([P, 1], f32, tag="z")
                relu = nc.scalar.activation(
                    w[:], scr[:], mybir.ActivationFunctionType.Relu, accum_out=zcol[:]
                )
                for dep in (prev_zeps, prev_res):
                    if dep is not None:
                        bass._add_dep_helper(
                            relu.ins, dep.ins, sync=False, reason="act order"
                        )

                # Z = sum over all t of relu'd scores.  Emitted before the
                # pooling matmuls: its single ACT wait (>= relu tick) also
                # covers the pooling matmuls' w/pout-release deps.
                pz = pszp.tile([1, 1], f32)
                mm_z = nc.tensor.matmul(
                    pz[:], zcol[:], ones_col[:], start=True, stop=True
                )
                bass._add_dep_helper(mm_z.ins, mm_b.ins, sync=False, reason="pe order")

                # pooled[d] = sum_t w_t * h[t, d] accumulated over chunks
                pout = psoutp.tile([1, D], f32)
                prev_mm = mm_z
                for c in range(TC):
                    mm_p = nc.tensor.matmul(
                        pout[:],
                        w[:, c : c + 1],
                        h_sb[:, c, :],
                        start=(c == 0),
                        stop=(c == TC - 1),
                    )
                    bass._add_dep_helper(
                        mm_p.ins, prev_mm.ins, sync=False, reason="pe order"
                    )
                    prev_mm = mm_p
                prev_pe = prev_mm

                zeps = smallp.tile([1, 1], f32, tag="zeps")
                prev_zeps = nc.scalar.activation(
                    zeps[:],
                    pz[:],
                    mybir.ActivationFunctionType.Identity,
                    bias=eps_tile[0:1, 0:1],
                )
                zrec = smallp.tile([1, 1], f32, tag="zrec")
                nc.vector.reciprocal(zrec[:], zeps[:])
                # scale on ACT so PSUM-slot releases are ACT-observed
                res = smallp.tile([1, D], f32, tag="res")
                prev_res = nc.scalar.mul(res[:], pout[:], zrec[0:1, 0:1])
                nc.sync.dma_start(out_ap[b : b + 1, :], res[:])

    nc.finalize()
    _nc_cache = nc
    return nc


def _run(h_all: np.ndarray, trace: bool = False):
    nc = _build()
    h_all = np.ascontiguousarray(np.asarray(h_all), dtype=np.float32)
    assert h_all.shape == (T, B, D)
    in_maps = [
        {"h": np.ascontiguousarray(h_all[:, c * BPC : (c + 1) * BPC, :])}
        for c in range(NCORES)
    ]
    r = run_bass_kernel_spmd(nc, in_maps, list(range(NCORES)), trace=trace)
    out = np.concatenate([r.results[c]["out"] for c in range(NCORES)], axis=0)
    return out, r


def kernel(h_all: np.ndarray, xin: np.ndarray | None = None) -> np.ndarray:
    out, _ = _run(h_all)
    return out


# revision 16
# speedup vs baseline: 1.5278x; 1.5278x over previous
"""AttentivePooling Trainium2 kernel.

Reference semantics (h_all: [T, B, D] f32, xin unused):
    h_last = h_all[-1]                       # [B, D]
    a[b, t] = <h_all[t, b, :], h_last[b, :]> / sqrt(D)
    r = relu(a)
    w = r / (sum_t r + 1e-9)
    out[b, d] = sum_t w[b, t] * h_all[t, b, d]

Strategy: data-parallel over B across 8 cores (8 batches/core, no
collectives).  Per batch on-device:
  - one 4MB SWDGE DMA loads h_b as 16 SBUF chunks [128(t), 512(d)]
    (t = c*128 + p); a single HWDGE dma_start with 2048 descriptors
    wedges the exec unit, SWDGE handles it
  - PE broadcasts h_last across 128 partitions into PSUM
    (ones[1,128].T @ hl[1,512]); the DVE multiply reads it from PSUM
  - scores: DVE tensor_tensor multiply; the free-dim reduction is
    split between ACT (activation accum_out, scale=1/sqrt(D) folded)
    and DVE (tensor_reduce + tensor_scalar rescale) to balance load.
    (The fused DVE tensor_tensor_reduce crashes the exec unit on HW.)
  - ACT relu with accum_out produces weights + their per-partition sums
  - PE accumulates sum_t w_t * h_t into PSUM [1, 512] (w stationary)
  - PE reduces the weight-sum across partitions via a ones column
  - DVE computes 1/(Z + 1e-9); ACT scales the pooled vector
"""

import numpy as np
from contextlib import ExitStack

import concourse.bass as bass
import concourse.tile as tile
from concourse import bacc, mybir
from concourse.bass_utils import run_bass_kernel_spmd

T, B, D = 2048, 64, 512
NCORES = 8
BPC = B // NCORES  # batches per core
P = 128
TC = T // P  # 16 T-chunks per batch
SCALE = float(1.0 / np.sqrt(np.float32(D)))
N_ACT_REDUCE = 10  # chunks whose reduction runs on ACT; rest on DVE

_nc_cache = None


def _build():
    global _nc_cache
    if _nc_cache is not None:
        return _nc_cache
    nc = bacc.Bacc("TRN2", debug=False, target_bir_lowering=False, num_devices=NCORES)
    h = nc.dram_tensor("h", [T, BPC, D], mybir.dt.float32, kind="ExternalInput")
    out = nc.dram_tensor("out", [BPC, D], mybir.dt.float32, kind="ExternalOutput")
    h_ap = h.ap()
    out_ap = out.ap()
    f32 = mybir.dt.float32

    with tile.TileContext(nc) as tc:
        with ExitStack() as ctx:
            hpool = ctx.enter_context(tc.tile_pool(name="h", bufs=3))
            tmpp = ctx.enter_context(tc.tile_pool(name="tmp", bufs=4))
            smallp = ctx.enter_context(tc.tile_pool(name="small", bufs=3))
            constp = ctx.enter_context(tc.tile_pool(name="const", bufs=1))
            psbcp = ctx.enter_context(tc.tile_pool(name="psb", bufs=2, space="PSUM"))
            psoutp = ctx.enter_context(tc.tile_pool(name="pso", bufs=3, space="PSUM"))
            pszp = ctx.enter_context(tc.tile_pool(name="psz", bufs=3, space="PSUM"))

            ones_row = constp.tile([1, P], f32)
            nc.vector.memset(ones_row[:], 1.0)
            ones_col = constp.tile([P, 1], f32)
            nc.vector.memset(ones_col[:], 1.0)
            eps_tile = constp.tile([1, 1], f32)
            nc.vector.memset(eps_tile[:], 1e-9)
            # all h_last rows for this core's batches, on partition 0
            hl_flat = constp.tile([1, BPC * D], f32)
            nc.sync.dma_start(
                hl_flat[:], h_ap[T - 1 : T, :, :].rearrange("t b d -> t (b d)")
            )

            for b in range(BPC):
                h_sb = hpool.tile([P, TC, D], f32, tag="hsb")
                nc.gpsimd.dma_start(
                    h_sb[:], h_ap[:, b, :].rearrange("(c p) d -> p c d", p=P)
                )

                # broadcast h_last[b] to all 128 partitions (stays in PSUM;
                # DVE reads it via the PSUM port, freeing SBUF bandwidth)
                psb = psbcp.tile([P, D], f32, tag="psb")
                nc.tensor.matmul(
                    psb[:],
                    ones_row[:, :],
                    hl_flat[0:1, b * D : (b + 1) * D],
                    start=True,
                    stop=True,
                )

                # scores: scr[p, c] = sum_d h[t, d] * hl[d] * SCALE
                scr = smallp.tile([P, TC], f32, tag="scr")
                for c in range(TC):
                    tmp = tmpp.tile([P, D], f32, tag="tmp")
                    nc.vector.tensor_tensor(
                        tmp[:], h_sb[:, c, :], psb[:], mybir.AluOpType.mult
                    )
                    if c < N_ACT_REDUCE:
                        nc.scalar.activation(
                            tmp[:],
                            tmp[:],
                            mybir.ActivationFunctionType.Copy,
                            scale=SCALE,
                            accum_out=scr[:, c : c + 1],
                        )
                    else:
                        nc.vector.tensor_reduce(
                            scr[:, c : c + 1],
                            tmp[:],
                            mybir.AxisListType.X,
                            mybir.AluOpType.add,
                        )
                # rescale the DVE-reduced columns (ACT ones had SCALE folded)
                if N_ACT_REDUCE < TC:
                    nc.vector.tensor_scalar_mul(
                        scr[:, N_ACT_REDUCE:TC], scr[:, N_ACT_REDUCE:TC], SCALE
                    )

                # relu + per-partition sum of relu'd scores
                w = smallp.tile([P, TC], f32, tag="w")
                zcol = smallp.tile([P, 1], f32, tag="z")
                nc.scalar.activation(
                    w[:], scr[:], mybir.ActivationFunctionType.Relu, accum_out=zcol[:]
                )

                # pooled[d] = sum_t w_t * h[t, d] accumulated over chunks
                pout = psoutp.tile([1, D], f32)
                for c in range(TC):
                    nc.tensor.matmul(
                        pout[:],
                        w[:, c : c + 1],
                        h_sb[:, c, :],
                        start=(c == 0),
                        stop=(c == TC - 1),
                    )
                # Z = sum over all t of relu'd scores
                pz = pszp.tile([1, 1], f32)
                nc.tensor.matmul(pz[:], zcol[:], ones_col[:], start=True, stop=True)

                zeps = smallp.tile([1, 1], f32, tag="zeps")
                nc.scalar.activation(
                    zeps[:],
                    pz[:],
                    mybir.ActivationFunctionType.Identity,
                    bias=eps_tile[0:1, 0:1],
                )
                zrec = smallp.tile([1, 1], f32, tag="zrec")
                nc.vector.reciprocal(zrec[:], zeps[:])
                res = smallp.tile([1, D], f32, tag="res")
                nc.scalar.mul(res[:], pout[:], zrec[0:1, 0:1])
                nc.sync.dma_start(out_ap[b : b + 1, :], res[:])

    nc.finalize()
    _nc_cache = nc
    return nc


def _run(h_all: np.ndarray, trace: bool = False):
    nc = _build()
    h_all = np.ascontiguousarray(np.asarray(h_all), dtype=np.float32)
    assert h_all.shape == (T, B, D)
    in_maps = [
        {"h": np.ascontiguousarray(h_all[:, c * BPC : (c + 1) * BPC, :])}
        for c in range(NCORES)
    ]
    r = run_bass_kernel_spmd(nc, in_maps, list(range(NCORES)), trace=trace)
    out = np.concatenate([r.results[c]["out"] for c in range(NCORES)], axis=0)
    return out, r


def kernel(h_all: np.ndarray, xin: np.ndarray | None = None) -> np.ndarray:
    out, _ = _run(h_all)
    return out
